# revision 1
# baseline (speedup 1.0000x reference)
"""Two-layer GCN (BongardGNN) on 8 Trainium2 NeuronCores.

This toolchain has no usable data-dependent-addressing primitive (the
Anthropic extended DMA ucode is absent from the image and the walrus
DynamicDMA lowering is disabled, leaving indirect DMA at ~100us per
128-row descriptor batch), so the kernel is organised as three dense
device launches with host-side, index-only reshuffles between them:

  P1 (1 core):  deg  = rowsum(slot-validity mask)      [dense reduce]
                dis  = 1/sqrt(1+deg)                   [sqrt+recip]
                q0   = dis * x                         [elementwise]
  host: gather q0 rows into dst-sorted padded-CSR slots  mg1[n,d] = q0[src]
  P2 (8 cores): agg1 = sum_d mg1                       [dense reduce]
                s1   = dis*(agg1 + q0_local)
                h1T  = relu(W1^T s1^T + b1)            [PE]
                q2   = dis * (W2^T h1T)^T              [PE]
  host: gather q2 rows into slots                      mg2[n,d] = q2[src]
  P3 (8 cores): out  = dis*(sum_d mg2 + q2_local) + b2 [dense reduce]

Node range is split 8 ways by dst ownership (25000 nodes/core); each
core's slot table covers exactly its own nodes, so no collectives are
needed. All arithmetic (degree counting, normalisation, scaling, both
GCN layers) runs on device; the host only sorts/pads/gathers by the
static edge_index.
"""

import os
import sys
import types

import numpy as np
import concourse.bacc as bacc
import concourse.tile as tile
from concourse import mybir
from concourse.bass_utils import run_bass_kernel_spmd
from concourse.masks import make_identity

F32 = mybir.dt.float32
U8 = mybir.dt.uint8

TRACE = bool(os.environ.get("GNN_TRACE"))
LAST_EXEC_NS = []


def _enable_tracing():
    """Register the axon NTFF profile hook (absent from this image's antenv)
    and stub out the slow artifact upload. Test-time only (GNN_TRACE=1)."""
    if "antenv.axon_hooks" not in sys.modules:
        mod = types.ModuleType("antenv.axon_hooks")
        state = {}
        mod.set_axon_ntff_profile_hook = lambda h: state.update(h=h)
        mod.get_axon_ntff_profile_hook = lambda: state.get("h")
        sys.modules["antenv.axon_hooks"] = mod
        import antenv

        antenv.axon_hooks = mod
        sys.path.insert(0, "/root/.axon_site")
        from trn_agent_boot.trn_boot import _ntff_profile_via_ctypes

        mod.set_axon_ntff_profile_hook(
            _ntff_profile_via_ctypes("/opt/axon/libaxon_pjrt.so")
        )
    import concourse.bass_utils as bu

    bu.upload_artifacts = lambda tmpdir: "skipped"


def _run(nc, in_maps, core_ids):
    if TRACE:
        _enable_tracing()
        res = run_bass_kernel_spmd(nc, in_maps, core_ids=core_ids, trace=True)
        LAST_EXEC_NS.append(res.exec_time_ns)
        return res
    return run_bass_kernel_spmd(nc, in_maps, core_ids=core_ids)

N = 200000
NCORES = 8
NPC = 25000            # nodes per core
LP = 196               # local nodes per partition
NPC_PAD = 128 * LP     # 25088
GP = 1568              # global nodes per partition
N_PAD = 128 * GP       # 200704
D = 40                 # padded-CSR slots per node (max in-degree 36 + margin)
D0, D1, D2 = 16, 32, 2
CORE_IDS = list(range(NCORES))


def build_p1():
    """deg -> dis -> q0 on one core."""
    nc = bacc.Bacc("TRN2", target_bir_lowering=False, debug=False)
    vmask = nc.dram_tensor("vmask", [128, GP * D], U8, kind="ExternalInput")
    xbm = nc.dram_tensor("xbm", [128, GP * D0], F32, kind="ExternalInput")
    q0_d = nc.dram_tensor("q0", [128, GP * D0], F32, kind="ExternalOutput")
    dis_d = nc.dram_tensor("dis", [128, GP], F32, kind="ExternalOutput")

    with tile.TileContext(nc) as tc:
        with tc.tile_pool(name="pool", bufs=2) as pool, tc.tile_pool(
            name="cpool", bufs=1
        ) as cpool:
            dis = cpool.tile([128, GP], F32)
            NCH = 8
            KC = GP // NCH  # 196 nodes per partition per chunk
            for u in range(NCH):
                vm8 = pool.tile([128, KC * D], U8, tag="vm8")
                nc.sync.dma_start(
                    out=vm8[:], in_=vmask[:, u * KC * D:(u + 1) * KC * D]
                )
                vmf = pool.tile([128, KC * D], F32, tag="vmf")
                nc.vector.tensor_copy(out=vmf[:], in_=vm8[:])
                nc.vector.tensor_reduce(
                    out=dis[:, u * KC:(u + 1) * KC],
                    in_=vmf[:].rearrange("p (k d) -> p k d", d=D),
                    axis=mybir.AxisListType.X,
                    op=mybir.AluOpType.add,
                )
            nc.vector.tensor_scalar_add(dis[:], dis[:], 1.0)
            nc.scalar.activation(dis[:], dis[:], mybir.ActivationFunctionType.Sqrt)
            nc.vector.reciprocal(dis[:], dis[:])
            nc.sync.dma_start(out=dis_d[:], in_=dis[:])
            for u in range(NCH):
                xc = pool.tile([128, KC * D0], F32, tag="xc")
                nc.sync.dma_start(
                    out=xc[:], in_=xbm[:, u * KC * D0:(u + 1) * KC * D0]
                )
                q0c = pool.tile([128, KC * D0], F32, tag="q0c")
                nc.vector.tensor_tensor(
                    out=q0c[:].rearrange("p (k f) -> p k f", f=D0),
                    in0=xc[:].rearrange("p (k f) -> p k f", f=D0),
                    in1=dis[:, u * KC:(u + 1) * KC]
                    .rearrange("p (k o) -> p k o", o=1)
                    .to_broadcast([128, KC, D0]),
                    op=mybir.AluOpType.mult,
                )
                nc.sync.dma_start(
                    out=q0_d[:, u * KC * D0:(u + 1) * KC * D0], in_=q0c[:]
                )
    nc.compile()
    return nc


def build_p2():
    """agg1 -> s1 -> h1 -> q2 per core."""
    nc = bacc.Bacc("TRN2", target_bir_lowering=False, debug=False)
    mg1 = nc.dram_tensor("mg1", [128, LP * D * D0], F32, kind="ExternalInput")
    q0l = nc.dram_tensor("q0l", [128, LP * D0], F32, kind="ExternalInput")
    disl_i = nc.dram_tensor("disl", [128, LP], F32, kind="ExternalInput")
    w1 = nc.dram_tensor("w1", [D0, D1], F32, kind="ExternalInput")
    b1c = nc.dram_tensor("b1c", [D1, 1], F32, kind="ExternalInput")
    w2 = nc.dram_tensor("w2", [D1, D2], F32, kind="ExternalInput")
    q2l_d = nc.dram_tensor("q2l", [NPC_PAD, D2], F32, kind="ExternalOutput")
    q2raw = nc.dram_tensor("q2raw", [NPC_PAD, D2], F32)

    with tile.TileContext(nc) as tc:
        with (
            tc.tile_pool(name="pool", bufs=2) as pool,
            tc.tile_pool(name="cpool", bufs=1) as cpool,
            tc.tile_pool(name="psum", bufs=2, space="PSUM") as psum,
        ):
            # ---- slot reduction: agg1[n,f] = sum_d mg1[n,d,f] ----
            aggsb = cpool.tile([128, LP * D0], F32)
            NCH = 14
            KC = LP // NCH  # 14 k's per chunk
            for u in range(NCH):
                mgc = pool.tile([128, KC * D * D0], F32, tag="mgc")
                nc.sync.dma_start(
                    out=mgc[:],
                    in_=mg1[:, u * KC * D * D0:(u + 1) * KC * D * D0],
                )
                nc.vector.tensor_reduce(
                    out=aggsb[:, u * KC * D0:(u + 1) * KC * D0].rearrange(
                        "p (k f) -> p k f", f=D0
                    ),
                    in_=mgc[:].rearrange(
                        "p (k d f) -> p k f d", d=D, f=D0
                    ),
                    axis=mybir.AxisListType.X,
                    op=mybir.AluOpType.add,
                )
            # ---- s1 = dis * (agg1 + q0_local), augmented with dis row ----
            q0lsb = cpool.tile([128, LP * D0], F32)
            nc.sync.dma_start(out=q0lsb[:], in_=q0l[:])
            disl = cpool.tile([128, LP], F32)
            nc.sync.dma_start(out=disl[:], in_=disl_i[:])
            nc.vector.tensor_tensor(
                out=aggsb[:], in0=aggsb[:], in1=q0lsb[:], op=mybir.AluOpType.add
            )
            s1aug = cpool.tile([128, LP * D0], F32)
            nc.vector.tensor_tensor(
                out=s1aug[:].rearrange("p (k f) -> p k f", f=D0),
                in0=aggsb[:].rearrange("p (k f) -> p k f", f=D0),
                in1=disl[:]
                .rearrange("p (k o) -> p k o", o=1)
                .to_broadcast([128, LP, D0]),
                op=mybir.AluOpType.mult,
            )
            # ---- transpose to feature-major, then W1/W2 matmuls ----
            ident = cpool.tile([128, 128], F32)
            make_identity(nc, ident[:])
            w1sb = cpool.tile([D0, D1], F32)
            nc.sync.dma_start(out=w1sb[:], in_=w1[:])
            b1sb = cpool.tile([D1, 1], F32)
            nc.sync.dma_start(out=b1sb[:], in_=b1c[:])
            w2sb = cpool.tile([D1, D2], F32)
            nc.sync.dma_start(out=w2sb[:], in_=w2[:])
            with tc.tile_pool(name="epool", bufs=1) as epool:
                HP = 64
                HN = HP * LP  # 12544 nodes per half
                for h in range(2):
                    psl = slice(h * HP, (h + 1) * HP)
                    s1T = epool.tile([D0, HN], F32, tag="s1T")
                    for k in range(LP):
                        tp = psum.tile([D0, HP], F32, tag="tp")
                        nc.tensor.transpose(
                            out=tp[:],
                            in_=s1aug[psl, k * D0:(k + 1) * D0],
                            identity=ident[psl, psl],
                        )
                        # node-linear columns: col p*LP + k
                        nc.vector.tensor_copy(
                            out=s1T[:].rearrange("f (p k) -> f p k", k=LP)[
                                :, :, k
                            ],
                            in_=tp[:],
                        )
                    CH2 = 448  # 12544 = 28*448
                    for t in range(HN // CH2):
                        sl = slice(t * CH2, (t + 1) * CH2)
                        h1p = psum.tile([D1, CH2], F32, tag="h1p")
                        nc.tensor.matmul(
                            out=h1p[:],
                            lhsT=w1sb[:],
                            rhs=s1T[:D0, sl],
                            start=True,
                            stop=True,
                        )
                        h1s = pool.tile([D1, CH2], F32, tag="h1s")
                        nc.scalar.activation(
                            h1s[:],
                            h1p[:],
                            mybir.ActivationFunctionType.Relu,
                            bias=b1sb[:],
                        )
                        h2p = psum.tile([D2, CH2], F32, tag="h2p")
                        nc.tensor.matmul(
                            out=h2p[:], lhsT=w2sb[:], rhs=h1s[:],
                            start=True, stop=True,
                        )
                        q2c = pool.tile([D2, CH2], F32, tag="q2c")
                        nc.vector.tensor_copy(out=q2c[:], in_=h2p[:])
                        nc.sync.dma_start(
                            out=q2raw[
                                h * HN + t * CH2: h * HN + (t + 1) * CH2, :
                            ].rearrange("n f -> f n"),
                            in_=q2c[:],
                        )
            q2sb = cpool.tile([128, LP * D2], F32)
            nc.sync.dma_start(
                out=q2sb[:], in_=q2raw[:].rearrange("(p k) f -> p (k f)", p=128)
            )
            nc.vector.tensor_tensor(
                out=q2sb[:].rearrange("p (k f) -> p k f", f=D2),
                in0=q2sb[:].rearrange("p (k f) -> p k f", f=D2),
                in1=disl[:]
                .rearrange("p (k o) -> p k o", o=1)
                .to_broadcast([128, LP, D2]),
                op=mybir.AluOpType.mult,
            )
            nc.sync.dma_start(
                out=q2l_d[:].rearrange("(p k) f -> p (k f)", p=128), in_=q2sb[:]
            )
    nc.compile()
    return nc


def build_p3():
    """out = dis*(sum_d mg2 + q2_local) + b2 per core."""
    nc = bacc.Bacc("TRN2", target_bir_lowering=False, debug=False)
    mg2 = nc.dram_tensor("mg2", [128, LP * D * D2], F32, kind="ExternalInput")
    q2l = nc.dram_tensor("q2l", [128, LP * D2], F32, kind="ExternalInput")
    disl_i = nc.dram_tensor("disl", [128, LP], F32, kind="ExternalInput")
    b2r = nc.dram_tensor("b2r", [128, D2], F32, kind="ExternalInput")
    out2 = nc.dram_tensor("out2", [128, LP * D2], F32, kind="ExternalOutput")

    with tile.TileContext(nc) as tc:
        with tc.tile_pool(name="pool", bufs=2) as pool, tc.tile_pool(
            name="cpool", bufs=1
        ) as cpool:
            aggsb = cpool.tile([128, LP * D2], F32)
            NCH = 4
            KC = LP // NCH  # 49
            for u in range(NCH):
                mgc = pool.tile([128, KC * D * D2], F32, tag="mgc")
                nc.sync.dma_start(
                    out=mgc[:],
                    in_=mg2[:, u * KC * D * D2:(u + 1) * KC * D * D2],
                )
                nc.vector.tensor_reduce(
                    out=aggsb[:, u * KC * D2:(u + 1) * KC * D2].rearrange(
                        "p (k f) -> p k f", f=D2
                    ),
                    in_=mgc[:].rearrange(
                        "p (k d f) -> p k f d", d=D, f=D2
                    ),
                    axis=mybir.AxisListType.X,
                    op=mybir.AluOpType.add,
                )
            q2lsb = cpool.tile([128, LP * D2], F32)
            nc.sync.dma_start(out=q2lsb[:], in_=q2l[:])
            disl = cpool.tile([128, LP], F32)
            nc.sync.dma_start(out=disl[:], in_=disl_i[:])
            b2sb = cpool.tile([128, D2], F32)
            nc.sync.dma_start(out=b2sb[:], in_=b2r[:])
            nc.vector.tensor_tensor(
                out=aggsb[:], in0=aggsb[:], in1=q2lsb[:], op=mybir.AluOpType.add
            )
            nc.vector.tensor_tensor(
                out=aggsb[:].rearrange("p (k f) -> p k f", f=D2),
                in0=aggsb[:].rearrange("p (k f) -> p k f", f=D2),
                in1=disl[:]
                .rearrange("p (k o) -> p k o", o=1)
                .to_broadcast([128, LP, D2]),
                op=mybir.AluOpType.mult,
            )
            nc.vector.tensor_tensor(
                out=aggsb[:].rearrange("p (k f) -> p k f", f=D2),
                in0=aggsb[:].rearrange("p (k f) -> p k f", f=D2),
                in1=b2sb[:]
                .rearrange("p (o f) -> p o f", o=1)
                .to_broadcast([128, LP, D2]),
                op=mybir.AluOpType.add,
            )
            nc.sync.dma_start(out=out2[:], in_=aggsb[:])
    nc.compile()
    return nc


def build_slots(edge_index):
    """Host: dst-sorted padded-CSR slot structure. Returns per-core
    src-index tables [NPC_PAD, D] int32 (N_PAD-1 = zero-row sentinel) and
    the global slot-validity mask for the degree computation."""
    src = np.asarray(edge_index[0], np.int64)
    dst = np.asarray(edge_index[1], np.int64)
    order = np.argsort(dst, kind="stable")
    s_src = src[order].astype(np.int32)
    s_dst = dst[order]
    deg = np.bincount(s_dst, minlength=N).astype(np.int64)
    assert deg.max() <= D, f"max degree {deg.max()} exceeds D={D}"
    starts = np.zeros(N + 1, np.int64)
    np.cumsum(deg, out=starts[1:])
    # slot src table [N, D]: sentinel = last padded row (zero q-values)
    slot_src = np.full((N, D), N_PAD - 1, np.int32)
    pos = np.arange(len(s_src)) - starts[s_dst]
    slot_src[s_dst, pos] = s_src
    vmask = np.zeros((N_PAD, D), np.uint8)
    vmask[:N] = (np.arange(D)[None, :] < deg[:, None]).astype(np.uint8)
    return slot_src, vmask


def kernel(x, edge_index, W1, b1, W2, b2):
    LAST_EXEC_NS.clear()
    x = np.asarray(x, np.float32)
    W1 = np.asarray(W1, np.float32)
    b1 = np.asarray(b1, np.float32)
    W2 = np.asarray(W2, np.float32)
    b2 = np.asarray(b2, np.float32)

    slot_src, vmask = build_slots(edge_index)

    # ---- P1: deg/dis/q0 on core 0 ----
    xp = np.zeros((N_PAD, D0), np.float32)
    xp[:N] = x
    p1 = build_p1()
    r1 = _run(
        p1,
        [{
            "vmask": np.ascontiguousarray(vmask.reshape(128, GP * D)),
            "xbm": np.ascontiguousarray(xp.reshape(128, GP * D0)),
        }],
        core_ids=[0],
    ).results[0]
    q0 = r1["q0"].reshape(N_PAD, D0)
    dis = r1["dis"].reshape(N_PAD)

    # ---- host join 1: mg1 slots ----
    p2 = build_p2()
    in2 = []
    w1c = np.ascontiguousarray(W1)
    b1c = np.ascontiguousarray(b1.reshape(D1, 1))
    w2c = np.ascontiguousarray(W2)
    for c in range(NCORES):
        ss = np.full((NPC_PAD, D), N_PAD - 1, np.int32)
        ss[:NPC] = slot_src[c * NPC:(c + 1) * NPC]
        mg1 = q0[ss]                       # [NPC_PAD, D, D0]
        q0l = np.zeros((NPC_PAD, D0), np.float32)
        q0l[:NPC] = q0[c * NPC:(c + 1) * NPC]
        dl = np.zeros(NPC_PAD, np.float32)
        dl[:NPC] = dis[c * NPC:(c + 1) * NPC]
        in2.append(
            {
                "mg1": np.ascontiguousarray(mg1.reshape(128, LP * D * D0)),
                "q0l": np.ascontiguousarray(q0l.reshape(128, LP * D0)),
                "disl": np.ascontiguousarray(dl.reshape(128, LP)),
                "w1": w1c,
                "b1c": b1c,
                "w2": w2c,
            }
        )
    r2 = _run(p2, in2, core_ids=CORE_IDS).results
    q2g = np.zeros((N_PAD, D2), np.float32)
    for c in range(NCORES):
        q2g[c * NPC:(c + 1) * NPC] = r2[c]["q2l"][:NPC]

    # ---- host join 2: mg2 slots ----
    p3 = build_p3()
    b2r = np.ascontiguousarray(np.tile(b2.reshape(1, D2), (128, 1)))
    in3 = []
    for c in range(NCORES):
        ss = np.full((NPC_PAD, D), N_PAD - 1, np.int32)
        ss[:NPC] = slot_src[c * NPC:(c + 1) * NPC]
        mg2 = q2g[ss]                      # [NPC_PAD, D, D2]
        q2l = np.zeros((NPC_PAD, D2), np.float32)
        q2l[:NPC] = q2g[c * NPC:(c + 1) * NPC]
        dl = np.zeros(NPC_PAD, np.float32)
        dl[:NPC] = dis[c * NPC:(c + 1) * NPC]
        in3.append(
            {
                "mg2": np.ascontiguousarray(mg2.reshape(128, LP * D * D2)),
                "q2l": np.ascontiguousarray(q2l.reshape(128, LP * D2)),
                "disl": np.ascontiguousarray(dl.reshape(128, LP)),
                "b2r": b2r,
            }
        )
    r3 = _run(p3, in3, core_ids=CORE_IDS).results
    out = np.concatenate(
        [r3[c]["out2"].reshape(NPC_PAD, D2)[:NPC] for c in range(NCORES)], axis=0
    )
    return out.astype(np.float32)



# revision 13
# speedup vs baseline: 7.2643x; 7.2643x over previous
"""Two-layer GCN (BongardGNN) on 8 Trainium2 NeuronCores.

This toolchain has no usable data-dependent-addressing primitive, so the
kernel is organised as three dense device launches with host-side,
index-only reshuffles between them (the host never does arithmetic on
tensor values — it only sorts/pads/gathers/duplicates by the static
edge_index):

  P1 (8 cores): dis = 1/sqrt(1+deg)      [sqrt+recip]
                q0  = dis * x  -> bf16   [elementwise + downcast]
  host: gather q0 rows into degree-bucketed, feature-major CSR slots
  P2 (8 cores): agg1 = sum_d mg1        [dense bf16 reduce]
                s1   = dis*(agg1 + q0_local)
                h1   = relu(Wblk1 s1 + b1)   [block-diag PE matmul]
                q2   = dis * (Wblk2 h1) -> bf16
  host: gather q2 rows into slots
  P3 (8 cores): out  = dis*(sum_d mg2 + q2_local) + b2

Traffic-minimising choices vs a naive padded-CSR kernel:
  * messages stream in bf16 (device-side downcast; rel-err budget 2e-2)
  * nodes are globally degree-sorted and dealt round-robin over
    (core, partition-row), so per-row slot capacity = ceil2(max degree
    in row) and padding overhead is ~3% instead of 150%
  * the message gather is laid out feature-major (partition =
    group*16+feature), so the slot reduce lands directly in the matmul
    rhs layout: no PE transposes, and one block-diagonal weight matmul
    computes 4 node-groups at a time.
"""

import os
import sys
import types

import numpy as np
import ml_dtypes
import concourse.bacc as bacc
import concourse.tile as tile
from concourse import mybir
from concourse.bass_utils import run_bass_kernel_spmd

F32 = mybir.dt.float32
BF16 = mybir.dt.bfloat16
NPBF = ml_dtypes.bfloat16

TRACE = bool(os.environ.get("GNN_TRACE"))
LAST_EXEC_NS = []


def _enable_tracing():
    """Register the axon NTFF profile hook (absent from this image's antenv)
    and stub out the slow artifact upload. Test-time only (GNN_TRACE=1)."""
    if "antenv.axon_hooks" not in sys.modules:
        mod = types.ModuleType("antenv.axon_hooks")
        state = {}
        mod.set_axon_ntff_profile_hook = lambda h: state.update(h=h)
        mod.get_axon_ntff_profile_hook = lambda: state.get("h")
        sys.modules["antenv.axon_hooks"] = mod
        import antenv

        antenv.axon_hooks = mod
        sys.path.insert(0, "/root/.axon_site")
        from trn_agent_boot.trn_boot import _ntff_profile_via_ctypes

        mod.set_axon_ntff_profile_hook(
            _ntff_profile_via_ctypes("/opt/axon/libaxon_pjrt.so")
        )
    import concourse.bass_utils as bu

    bu.upload_artifacts = lambda tmpdir: "skipped"


def _run(nc, in_maps, core_ids):
    if TRACE:
        _enable_tracing()
        res = run_bass_kernel_spmd(nc, in_maps, core_ids=core_ids, trace=True)
        LAST_EXEC_NS.append(res.exec_time_ns)
        return res
    return run_bass_kernel_spmd(nc, in_maps, core_ids=core_ids)


N = 200000
NCORES = 8
D0, D1, D2 = 16, 32, 2
CORE_IDS = list(range(NCORES))

# P1 grid: contiguous 25000-node slices, node = p*P1K + k per core
NPC1 = N // NCORES     # 25000
P1K = 196
P1PAD = 128 * P1K      # 25088

# P2 grid: partition p = g*16 + f (g = node group, f = feature);
# per core K2 node columns per group; global rank r = k*64 + g*8 + c
K2 = 3136              # 7 blocks of 448; 64*K2 = 200704 >= N
BLK = 448
NBLK = K2 // BLK
N2PAD = 64 * K2        # 200704

# P3 grid: partition p = node lane; rank r = k*1024 + c*128 + p
K3 = 196               # 1024*K3 = 200704 >= N


def _ceil2(a):
    return ((a + 1) // 2) * 2


def _runs(caps):
    """Maximal (k0, k1, cap) runs of equal capacity."""
    runs = []
    k0 = 0
    for k in range(1, len(caps) + 1):
        if k == len(caps) or caps[k] != caps[k0]:
            runs.append((k0, k, int(caps[k0])))
            k0 = k
    return runs


def build_p1():
    """deg -> dis -> q0(bf16), 25088 nodes per core."""
    nc = bacc.Bacc("TRN2", target_bir_lowering=False, debug=False)
    xc = nc.dram_tensor("xc", [128, P1K * D0], F32, kind="ExternalInput")
    degc = nc.dram_tensor("degc", [128, P1K], F32, kind="ExternalInput")
    q0o = nc.dram_tensor("q0o", [128, P1K * D0], BF16, kind="ExternalOutput")
    diso = nc.dram_tensor("diso", [128, P1K], F32, kind="ExternalOutput")

    with tile.TileContext(nc) as tc:
        with tc.tile_pool(name="pool", bufs=2) as pool, tc.tile_pool(
            name="cpool", bufs=1
        ) as cpool:
            dis = cpool.tile([128, P1K], F32)
            nc.sync.dma_start(out=dis[:], in_=degc[:])
            nc.vector.tensor_scalar_add(dis[:], dis[:], 1.0)
            nc.scalar.activation(dis[:], dis[:], mybir.ActivationFunctionType.Sqrt)
            nc.vector.reciprocal(dis[:], dis[:])
            nc.sync.dma_start(out=diso[:], in_=dis[:])
            NCH = 2
            KC = P1K // NCH
            for u in range(NCH):
                xt = pool.tile([128, KC * D0], F32, tag="xt")
                nc.sync.dma_start(
                    out=xt[:], in_=xc[:, u * KC * D0:(u + 1) * KC * D0]
                )
                q0t = pool.tile([128, KC * D0], BF16, tag="q0t")
                nc.vector.tensor_tensor(
                    out=q0t[:].rearrange("p (k f) -> p k f", f=D0),
                    in0=xt[:].rearrange("p (k f) -> p k f", f=D0),
                    in1=dis[:, u * KC:(u + 1) * KC]
                    .rearrange("p (k o) -> p k o", o=1)
                    .to_broadcast([128, KC, D0]),
                    op=mybir.AluOpType.mult,
                )
                nc.sync.dma_start(
                    out=q0o[:, u * KC * D0:(u + 1) * KC * D0], in_=q0t[:]
                )
    nc.compile()
    return nc


def build_p2(caps2):
    """Slot reduce + both GCN matmuls, feature-major, per core."""
    coloff = np.zeros(K2 + 1, np.int64)
    np.cumsum(caps2, out=coloff[1:])
    S = int(coloff[K2])

    nc = bacc.Bacc("TRN2", target_bir_lowering=False, debug=False)
    mg1 = nc.dram_tensor("mg1", [128, S], BF16, kind="ExternalInput")
    q0l = nc.dram_tensor("q0l", [128, K2], BF16, kind="ExternalInput")
    disf = nc.dram_tensor("disf", [128, K2], F32, kind="ExternalInput")
    disq = nc.dram_tensor("disq", [8, 2 * K2], F32, kind="ExternalInput")
    w1d = nc.dram_tensor("w1d", [128, 128], F32, kind="ExternalInput")
    w2d = nc.dram_tensor("w2d", [128, 8], F32, kind="ExternalInput")
    b1d = nc.dram_tensor("b1d", [128, 1], F32, kind="ExternalInput")
    q2d = nc.dram_tensor("q2d", [8, 2 * K2], BF16, kind="ExternalOutput")

    with tile.TileContext(nc) as tc:
        with (
            tc.tile_pool(name="pool", bufs=2) as pool,
            tc.tile_pool(name="cpool", bufs=1) as cpool,
            tc.tile_pool(name="psum", bufs=2, space="PSUM") as psum,
        ):
            disfs = cpool.tile([128, K2], F32)
            nc.sync.dma_start(out=disfs[:], in_=disf[:])
            q0ls = cpool.tile([128, K2], BF16)
            nc.sync.dma_start(out=q0ls[:], in_=q0l[:])
            disqs = cpool.tile([8, 2 * K2], F32)
            nc.sync.dma_start(out=disqs[:], in_=disq[:])
            w1f = cpool.tile([128, 128], F32)
            nc.sync.dma_start(out=w1f[:], in_=w1d[:])
            w1b = cpool.tile([128, 128], BF16)
            nc.vector.tensor_copy(out=w1b[:], in_=w1f[:])
            w2f = cpool.tile([128, 8], F32)
            nc.sync.dma_start(out=w2f[:], in_=w2d[:])
            w2b = cpool.tile([128, 8], BF16)
            nc.vector.tensor_copy(out=w2b[:], in_=w2f[:])
            b1s = cpool.tile([128, 1], F32)
            nc.sync.dma_start(out=b1s[:], in_=b1d[:])
            q2sb = cpool.tile([8, 2 * K2], BF16)

            for blk in range(NBLK):
                kb0, kb1 = blk * BLK, (blk + 1) * BLK
                c0, c1 = int(coloff[kb0]), int(coloff[kb1])
                mgt = pool.tile([128, c1 - c0], BF16, tag="mgt")
                nc.sync.dma_start(out=mgt[:], in_=mg1[:, c0:c1])
                agg = pool.tile([128, BLK], F32, tag="agg")
                for k0, k1, cap in _runs(caps2[kb0:kb1]):
                    o0 = int(coloff[kb0 + k0]) - c0
                    o1 = int(coloff[kb0 + k1]) - c0
                    nc.vector.tensor_reduce(
                        out=agg[:, k0:k1],
                        in_=mgt[:, o0:o1].rearrange("p (k d) -> p k d", d=cap),
                        axis=mybir.AxisListType.X,
                        op=mybir.AluOpType.add,
                    )
                # s1 = disf * (agg + q0l)
                q0f = pool.tile([128, BLK], F32, tag="q0f")
                nc.vector.tensor_copy(out=q0f[:], in_=q0ls[:, kb0:kb1])
                nc.vector.tensor_tensor(
                    out=agg[:], in0=agg[:], in1=q0f[:], op=mybir.AluOpType.add
                )
                s1b = pool.tile([128, BLK], BF16, tag="s1b")
                nc.vector.tensor_tensor(
                    out=s1b[:],
                    in0=agg[:],
                    in1=disfs[:, kb0:kb1],
                    op=mybir.AluOpType.mult,
                )
                for h in (0, 1):
                    ps1 = psum.tile([128, BLK], F32, tag="ps1")
                    nc.tensor.matmul(
                        out=ps1[:],
                        lhsT=w1b[64 * h:64 * h + 64, :],
                        rhs=s1b[64 * h:64 * h + 64, :],
                        start=True,
                        stop=True,
                    )
                    h1s = pool.tile([128, BLK], BF16, tag="h1s")
                    nc.scalar.activation(
                        h1s[:],
                        ps1[:],
                        mybir.ActivationFunctionType.Relu,
                        bias=b1s[:],
                    )
                    ps2 = psum.tile([8, BLK], F32, tag="ps2")
                    nc.tensor.matmul(
                        out=ps2[:], lhsT=w2b[:], rhs=h1s[:], start=True, stop=True
                    )
                    nc.vector.tensor_tensor(
                        out=q2sb[:, h * K2 + kb0:h * K2 + kb1],
                        in0=ps2[:],
                        in1=disqs[:, h * K2 + kb0:h * K2 + kb1],
                        op=mybir.AluOpType.mult,
                    )
            nc.sync.dma_start(out=q2d[:], in_=q2sb[:])
    nc.compile()
    return nc


def build_p3(caps3):
    """out = dis*(sum_d mg2 + q2_local) + b2 per core."""
    coloff = np.zeros(K3 + 1, np.int64)
    np.cumsum(caps3, out=coloff[1:])
    S3 = int(coloff[K3])

    nc = bacc.Bacc("TRN2", target_bir_lowering=False, debug=False)
    mg2 = nc.dram_tensor("mg2", [128, 2 * S3], BF16, kind="ExternalInput")
    q2l3 = nc.dram_tensor("q2l3", [128, K3 * D2], BF16, kind="ExternalInput")
    disl3 = nc.dram_tensor("disl3", [128, K3], F32, kind="ExternalInput")
    b2r = nc.dram_tensor("b2r", [128, D2], F32, kind="ExternalInput")
    out3 = nc.dram_tensor("out3", [128, K3 * D2], F32, kind="ExternalOutput")

    with tile.TileContext(nc) as tc:
        with tc.tile_pool(name="pool", bufs=2) as pool, tc.tile_pool(
            name="cpool", bufs=1
        ) as cpool:
            q2ls = cpool.tile([128, K3 * D2], BF16)
            nc.sync.dma_start(out=q2ls[:], in_=q2l3[:])
            disls = cpool.tile([128, K3], F32)
            nc.sync.dma_start(out=disls[:], in_=disl3[:])
            b2s = cpool.tile([128, D2], F32)
            nc.sync.dma_start(out=b2s[:], in_=b2r[:])
            NCH = 2
            KC = K3 // NCH
            for u in range(NCH):
                ku0, ku1 = u * KC, (u + 1) * KC
                c0, c1 = 2 * int(coloff[ku0]), 2 * int(coloff[ku1])
                mgt = pool.tile([128, c1 - c0], BF16, tag="mgt")
                nc.sync.dma_start(out=mgt[:], in_=mg2[:, c0:c1])
                agg = pool.tile([128, KC * D2], F32, tag="agg")
                for k0, k1, cap in _runs(caps3[ku0:ku1]):
                    o0 = 2 * int(coloff[ku0 + k0]) - c0
                    o1 = 2 * int(coloff[ku0 + k1]) - c0
                    nc.vector.tensor_reduce(
                        out=agg[:, k0 * D2:k1 * D2],
                        in_=mgt[:, o0:o1].rearrange(
                            "p (k f d) -> p k f d", f=D2, d=cap
                        ),
                        axis=mybir.AxisListType.X,
                        op=mybir.AluOpType.add,
                    )
                q2f = pool.tile([128, KC * D2], F32, tag="q2f")
                nc.vector.tensor_copy(
                    out=q2f[:], in_=q2ls[:, ku0 * D2:ku1 * D2]
                )
                nc.vector.tensor_tensor(
                    out=agg[:], in0=agg[:], in1=q2f[:], op=mybir.AluOpType.add
                )
                nc.vector.tensor_tensor(
                    out=agg[:].rearrange("p (k f) -> p k f", f=D2),
                    in0=agg[:].rearrange("p (k f) -> p k f", f=D2),
                    in1=disls[:, ku0:ku1]
                    .rearrange("p (k o) -> p k o", o=1)
                    .to_broadcast([128, KC, D2]),
                    op=mybir.AluOpType.mult,
                )
                nc.vector.tensor_tensor(
                    out=agg[:].rearrange("p (k f) -> p k f", f=D2),
                    in0=agg[:].rearrange("p (k f) -> p k f", f=D2),
                    in1=b2s[:]
                    .rearrange("p (o f) -> p o f", o=1)
                    .to_broadcast([128, KC, D2]),
                    op=mybir.AluOpType.add,
                )
                nc.sync.dma_start(
                    out=out3[:, ku0 * D2:ku1 * D2], in_=agg[:]
                )
    nc.compile()
    return nc


def kernel(x, edge_index, W1, b1, W2, b2):
    LAST_EXEC_NS.clear()
    x = np.asarray(x, np.float32)
    W1 = np.asarray(W1, np.float32)
    b1 = np.asarray(b1, np.float32)
    W2 = np.asarray(W2, np.float32)
    b2 = np.asarray(b2, np.float32)
    src = np.asarray(edge_index[0], np.int64)
    dst = np.asarray(edge_index[1], np.int64)

    # ---- host index prep: dst-sorted slot table, degree-sorted deal ----
    deg = np.bincount(dst, minlength=N).astype(np.int64)
    capmax = max(int(_ceil2(int(deg.max()))), 2)
    order_e = np.argsort(dst, kind="stable")
    s_src = src[order_e]
    s_dst = dst[order_e]
    starts = np.zeros(N + 1, np.int64)
    np.cumsum(deg, out=starts[1:])
    slot = np.full((N + 1, capmax), N, np.int32)  # row N = sentinel
    pos = np.arange(len(s_src)) - starts[s_dst]
    slot[s_dst, pos] = s_src

    onode = np.argsort(-deg, kind="stable")
    order_ext = np.concatenate([onode, np.full(N2PAD - N, N, np.int64)])
    deg_ext = np.concatenate([deg[onode], np.zeros(N2PAD - N, np.int64)])
    caps2 = np.maximum(_ceil2(deg_ext[::64]), 2).astype(np.int64)    # [K2]
    caps3 = np.maximum(_ceil2(deg_ext[::1024]), 2).astype(np.int64)  # [K3]
    nodes2 = order_ext.reshape(K2, 8, NCORES)    # [k, g, c]
    nodes3 = order_ext.reshape(K3, NCORES, 128)  # [k, c, p]
    runs2 = _runs(caps2)
    runs3 = _runs(caps3)
    coloff2 = np.zeros(K2 + 1, np.int64)
    np.cumsum(caps2, out=coloff2[1:])
    S2 = int(coloff2[K2])
    coloff3 = np.zeros(K3 + 1, np.int64)
    np.cumsum(caps3, out=coloff3[1:])
    S3 = int(coloff3[K3])

    # ---- P1: dis + q0(bf16) on 8 cores ----
    p1 = build_p1()
    in1 = []
    for c in range(NCORES):
        xp = np.zeros((P1PAD, D0), np.float32)
        xp[:NPC1] = x[c * NPC1:(c + 1) * NPC1]
        dg = np.zeros(P1PAD, np.float32)
        dg[:NPC1] = deg[c * NPC1:(c + 1) * NPC1]
        in1.append(
            {
                "xc": np.ascontiguousarray(xp.reshape(128, P1K * D0)),
                "degc": np.ascontiguousarray(dg.reshape(128, P1K)),
            }
        )
    r1 = _run(p1, in1, core_ids=CORE_IDS).results
    q0ext = np.zeros((N + 1, D0), NPBF)
    dis = np.zeros(N + 1, np.float32)
    for c in range(NCORES):
        q0ext[c * NPC1:(c + 1) * NPC1] = np.asarray(r1[c]["q0o"]).reshape(
            P1PAD, D0
        )[:NPC1]
        dis[c * NPC1:(c + 1) * NPC1] = np.asarray(r1[c]["diso"]).reshape(P1PAD)[
            :NPC1
        ]

    # ---- host join 1: feature-major bucketed mg1 slots ----
    p2 = build_p2(caps2)
    w1blk = np.zeros((128, 128), np.float32)
    w2blk = np.zeros((128, 8), np.float32)
    for g in range(4):
        w1blk[16 * g:16 * g + 16, 32 * g:32 * g + 32] = W1
        w1blk[64 + 16 * g:64 + 16 * g + 16, 32 * g:32 * g + 32] = W1
        w2blk[32 * g:32 * g + 32, 2 * g:2 * g + 2] = W2
    b1blk = np.ascontiguousarray(np.tile(b1, 4).reshape(128, 1))
    in2 = []
    for c in range(NCORES):
        grid = nodes2[:, :, c].T  # [8, K2]
        idx = np.empty((8, S2), np.int32)
        for k0, k1, cap in runs2:
            idx[:, coloff2[k0]:coloff2[k1]] = slot[
                grid[:, k0:k1], :cap
            ].reshape(8, -1)
        mg1 = np.ascontiguousarray(
            q0ext[idx].transpose(0, 2, 1)
        ).reshape(128, S2)
        q0lc = np.ascontiguousarray(
            q0ext[grid].transpose(0, 2, 1)
        ).reshape(128, K2)
        disg = dis[grid]  # [8, K2]
        disfc = np.ascontiguousarray(
            np.repeat(disg[:, None, :], 16, axis=1)
        ).reshape(128, K2)
        # disq rows r = 2*g' + j, cols h*K2 + k hold dis(node(4h+g', k))
        disqc = np.ascontiguousarray(
            np.repeat(
                disg.reshape(2, 4, 1, K2).transpose(1, 2, 0, 3), 2, axis=1
            ).reshape(8, 2 * K2)
        )
        in2.append(
            {
                "mg1": mg1,
                "q0l": q0lc,
                "disf": disfc,
                "disq": disqc,
                "w1d": w1blk,
                "w2d": w2blk,
                "b1d": b1blk,
            }
        )
    r2 = _run(p2, in2, core_ids=CORE_IDS).results
    q2ext = np.zeros((N + 1, D2), NPBF)
    for c in range(NCORES):
        vals = (
            np.asarray(r2[c]["q2d"])
            .reshape(4, 2, 2, K2)       # [g', j, h, k]
            .transpose(2, 0, 3, 1)      # [h, g', k, j]
            .reshape(8, K2, 2)
        )
        q2ext[nodes2[:, :, c].T] = vals  # [8, K2, 2]
    q2ext[N] = 0

    # ---- host join 2: mg2 slots ----
    p3 = build_p3(caps3)
    b2r = np.ascontiguousarray(np.tile(b2.reshape(1, D2), (128, 1)))
    in3 = []
    for c in range(NCORES):
        grid = nodes3[:, c, :].T  # [128, K3]
        mg2 = np.empty((128, 2 * S3), NPBF)
        for k0, k1, cap in runs3:
            g = q2ext[slot[grid[:, k0:k1], :cap]]  # [128, L, cap, 2]
            mg2[:, 2 * coloff3[k0]:2 * coloff3[k1]] = g.transpose(
                0, 1, 3, 2
            ).reshape(128, -1)
        q2lc = np.ascontiguousarray(q2ext[grid]).reshape(128, K3 * D2)
        dislc = np.ascontiguousarray(dis[grid])
        in3.append(
            {"mg2": mg2, "q2l3": q2lc, "disl3": dislc, "b2r": b2r}
        )
    r3 = _run(p3, in3, core_ids=CORE_IDS).results
    outfull = np.zeros((N + 1, D2), np.float32)
    for c in range(NCORES):
        outfull[nodes3[:, c, :].T] = np.asarray(r3[c]["out3"]).reshape(
            128, K3, D2
        )
    return np.ascontiguousarray(outfull[:N])


# revision 16
# speedup vs baseline: 10.0180x; 1.3791x over previous
"""Two-layer GCN (BongardGNN) on 8 Trainium2 NeuronCores.

This toolchain has no usable data-dependent-addressing primitive, so the
kernel is organised as three dense device launches with host-side,
index-only reshuffles between them (the host never does arithmetic on
tensor values — it only sorts/pads/gathers/duplicates by the static
edge_index):

  P1 (8 cores): dis = 1/sqrt(1+deg); q0 = dis*x -> bf16 (+ dis in bf16)
  host: gather q0 rows into degree-bucketed, feature-major CSR slots
        (self-loop included as slot 0, so no separate self term)
  P2 (8 cores): agg1 = sum_d mg1        [bf16 halving-tree adds]
                s1   = disf * agg1      [bf16]
                h1   = relu(Wblk1 s1 + b1)   [block-diag PE matmul]
                q2   = disq * (Wblk2 h1) -> bf16
  host: gather q2 rows into slots
  P3 (8 cores): out  = dis*(sum_d mg2) + b2

Performance structure (per core, memory-regime):
  * messages stream in bf16; nodes globally degree-sorted and dealt
    round-robin over (column, group, core) so per-column slot capacity
    = ceil2(max degree+1 in that 64-node row): ~3% padding
  * the slot reduce is a halving tree of CONTIGUOUS bf16 tensor_tensor
    adds (DVE packed mode, ~2 elem/cycle) over d-major slot planes —
    ~2x faster than the segmented tensor_reduce path; adjacent
    capacity-runs are merged when the padding cost is smaller than the
    per-instruction overhead
  * feature-major gather layout (partition = group*16+feature) means
    the reduce output lands directly in matmul-rhs layout: no PE
    transposes; one block-diagonal weight matmul serves 4 node groups.
"""

import os
import sys
import types

import numpy as np
import ml_dtypes
import concourse.bacc as bacc
import concourse.tile as tile
from concourse import mybir
from concourse.bass_utils import run_bass_kernel_spmd

F32 = mybir.dt.float32
BF16 = mybir.dt.bfloat16
NPBF = ml_dtypes.bfloat16

TRACE = bool(os.environ.get("GNN_TRACE"))
LAST_EXEC_NS = []


def _enable_tracing():
    """Register the axon NTFF profile hook (absent from this image's antenv)
    and stub out the slow artifact upload. Test-time only (GNN_TRACE=1)."""
    if "antenv.axon_hooks" not in sys.modules:
        mod = types.ModuleType("antenv.axon_hooks")
        state = {}
        mod.set_axon_ntff_profile_hook = lambda h: state.update(h=h)
        mod.get_axon_ntff_profile_hook = lambda: state.get("h")
        sys.modules["antenv.axon_hooks"] = mod
        import antenv

        antenv.axon_hooks = mod
        sys.path.insert(0, "/root/.axon_site")
        from trn_agent_boot.trn_boot import _ntff_profile_via_ctypes

        mod.set_axon_ntff_profile_hook(
            _ntff_profile_via_ctypes("/opt/axon/libaxon_pjrt.so")
        )
    import concourse.bass_utils as bu

    bu.upload_artifacts = lambda tmpdir: "skipped"


def _run(nc, in_maps, core_ids):
    if TRACE:
        _enable_tracing()
        res = run_bass_kernel_spmd(nc, in_maps, core_ids=core_ids, trace=True)
        LAST_EXEC_NS.append(res.exec_time_ns)
        return res
    return run_bass_kernel_spmd(nc, in_maps, core_ids=core_ids)


N = 200000
NCORES = 8
D0, D1, D2 = 16, 32, 2
CORE_IDS = list(range(NCORES))

# P1 grid: contiguous 25000-node slices, node = p*P1K + k per core
NPC1 = N // NCORES     # 25000
P1K = 196
P1PAD = 128 * P1K      # 25088

# P2 grid: partition p = g*16 + f (g = node group, f = feature);
# per core K2 node columns per group; global rank r = k*64 + g*8 + c
K2 = 3136              # 7 blocks of 448; 64*K2 = 200704 >= N
BLK = 448
NBLK = K2 // BLK
N2PAD = 64 * K2        # 200704

# P3 grid: partition p = node lane; rank r = k*1024 + c*128 + p
K3 = 196               # 1024*K3 = 200704 >= N


def _ceil2(a):
    return ((a + 1) // 2) * 2


def _runs(caps):
    """Maximal (k0, k1, cap) runs of equal capacity."""
    runs = []
    k0 = 0
    for k in range(1, len(caps) + 1):
        if k == len(caps) or caps[k] != caps[k0]:
            runs.append((k0, k, int(caps[k0])))
            k0 = k
    return runs


def _merged_runs(caps, max_extra=768):
    """Runs of equal cap, greedily merging a run into its (higher-cap)
    predecessor when the extra padded slots cost less than the saved
    per-instruction overhead."""
    runs = _runs(caps)
    out = [list(runs[0])]
    for k0, k1, v in runs[1:]:
        p0, p1, pv = out[-1]
        if (k1 - k0) * (pv - v) <= max_extra:
            out[-1][1] = k1
        else:
            out.append([k0, k1, v])
    return [(a, b, v) for a, b, v in out]


def build_p1():
    """deg -> dis (f32+bf16) -> q0(bf16), 25088 nodes per core."""
    nc = bacc.Bacc("TRN2", target_bir_lowering=False, debug=False)
    xc = nc.dram_tensor("xc", [128, P1K * D0], F32, kind="ExternalInput")
    degc = nc.dram_tensor("degc", [128, P1K], F32, kind="ExternalInput")
    q0o = nc.dram_tensor("q0o", [128, P1K * D0], BF16, kind="ExternalOutput")
    diso = nc.dram_tensor("diso", [128, P1K], F32, kind="ExternalOutput")
    disbo = nc.dram_tensor("disbo", [128, P1K], BF16, kind="ExternalOutput")

    with tile.TileContext(nc) as tc:
        with tc.tile_pool(name="pool", bufs=2) as pool, tc.tile_pool(
            name="cpool", bufs=1
        ) as cpool:
            dis = cpool.tile([128, P1K], F32)
            nc.sync.dma_start(out=dis[:], in_=degc[:])
            nc.vector.tensor_scalar_add(dis[:], dis[:], 1.0)
            nc.scalar.activation(dis[:], dis[:], mybir.ActivationFunctionType.Sqrt)
            nc.vector.reciprocal(dis[:], dis[:])
            nc.sync.dma_start(out=diso[:], in_=dis[:])
            disb = cpool.tile([128, P1K], BF16)
            nc.vector.tensor_copy(out=disb[:], in_=dis[:])
            nc.sync.dma_start(out=disbo[:], in_=disb[:])
            NCH = 2
            KC = P1K // NCH
            for u in range(NCH):
                xt = pool.tile([128, KC * D0], F32, tag="xt")
                nc.sync.dma_start(
                    out=xt[:], in_=xc[:, u * KC * D0:(u + 1) * KC * D0]
                )
                q0t = pool.tile([128, KC * D0], BF16, tag="q0t")
                nc.vector.tensor_tensor(
                    out=q0t[:].rearrange("p (k f) -> p k f", f=D0),
                    in0=xt[:].rearrange("p (k f) -> p k f", f=D0),
                    in1=dis[:, u * KC:(u + 1) * KC]
                    .rearrange("p (k o) -> p k o", o=1)
                    .to_broadcast([128, KC, D0]),
                    op=mybir.AluOpType.mult,
                )
                nc.sync.dma_start(
                    out=q0o[:, u * KC * D0:(u + 1) * KC * D0], in_=q0t[:]
                )
    nc.compile()
    return nc


def build_p2(runs2):
    """Tree slot-reduce + both GCN matmuls, feature-major, per core.

    mg1 column layout, per run (k0, k1, v): d-major slot planes —
    column off_r + d*L + (k-k0) holds q0[slot[node(g,k), d], f] at
    partition g*16+f. The reduce is an in-place halving tree of
    contiguous bf16 adds; the final level writes aggb, then
    s1 = aggb*disf feeds the block-diagonal matmuls.
    """
    offs = []
    off = 0
    for k0, k1, v in runs2:
        offs.append(off)
        off += (k1 - k0) * v
    smg = off
    mgt_max = max((k1 - k0) * v for k0, k1, v in runs2)

    nc = bacc.Bacc("TRN2", target_bir_lowering=False, debug=False)
    mg1 = nc.dram_tensor("mg1", [128, smg], BF16, kind="ExternalInput")
    disf = nc.dram_tensor("disf", [128, K2], BF16, kind="ExternalInput")
    disq = nc.dram_tensor("disq", [8, 2 * K2], F32, kind="ExternalInput")
    w1d = nc.dram_tensor("w1d", [128, 128], F32, kind="ExternalInput")
    w2d = nc.dram_tensor("w2d", [128, 8], F32, kind="ExternalInput")
    b1d = nc.dram_tensor("b1d", [128, 1], F32, kind="ExternalInput")
    q2d = nc.dram_tensor("q2d", [8, 2 * K2], BF16, kind="ExternalOutput")

    with tile.TileContext(nc) as tc:
        with (
            tc.tile_pool(name="pool", bufs=2) as pool,
            tc.tile_pool(name="cpool", bufs=1) as cpool,
            tc.tile_pool(name="psum", bufs=2, space="PSUM") as psum,
        ):
            disfs = cpool.tile([128, K2], BF16)
            nc.sync.dma_start(out=disfs[:], in_=disf[:])
            disqs = cpool.tile([8, 2 * K2], F32)
            nc.sync.dma_start(out=disqs[:], in_=disq[:])
            w1f = cpool.tile([128, 128], F32)
            nc.sync.dma_start(out=w1f[:], in_=w1d[:])
            w1b = cpool.tile([128, 128], BF16)
            nc.vector.tensor_copy(out=w1b[:], in_=w1f[:])
            w2f = cpool.tile([128, 8], F32)
            nc.sync.dma_start(out=w2f[:], in_=w2d[:])
            w2b = cpool.tile([128, 8], BF16)
            nc.vector.tensor_copy(out=w2b[:], in_=w2f[:])
            b1s = cpool.tile([128, 1], F32)
            nc.sync.dma_start(out=b1s[:], in_=b1d[:])
            s1b = cpool.tile([128, K2], BF16)
            q2sb = cpool.tile([8, 2 * K2], BF16)

            # ---- slot reduce: in-place bf16 halving tree per run ----
            for (k0, k1, v), off in zip(runs2, offs):
                L = k1 - k0
                cur = L * v
                mgt = pool.tile([128, mgt_max], BF16, tag="mgt")
                nc.sync.dma_start(out=mgt[:, :cur], in_=mg1[:, off:off + cur])
                vc = v
                while vc > 2:
                    if vc % 2:
                        # fold the odd top plane into plane 0 first
                        nc.vector.tensor_tensor(
                            out=mgt[:, :L],
                            in0=mgt[:, :L],
                            in1=mgt[:, (vc - 1) * L:vc * L],
                            op=mybir.AluOpType.add,
                        )
                        vc -= 1
                    h = vc // 2
                    if h == 1:
                        break
                    nc.vector.tensor_tensor(
                        out=mgt[:, :h * L],
                        in0=mgt[:, :h * L],
                        in1=mgt[:, h * L:2 * h * L],
                        op=mybir.AluOpType.add,
                    )
                    vc = h
                # final level: add the two remaining planes, scale by disf
                nc.vector.tensor_tensor(
                    out=mgt[:, :L],
                    in0=mgt[:, :L],
                    in1=mgt[:, L:2 * L],
                    op=mybir.AluOpType.add,
                )
                nc.vector.tensor_tensor(
                    out=s1b[:, k0:k1],
                    in0=mgt[:, :L],
                    in1=disfs[:, k0:k1],
                    op=mybir.AluOpType.mult,
                )

            # ---- block-diagonal matmuls per 448-column block ----
            for blk in range(NBLK):
                kb0, kb1 = blk * BLK, (blk + 1) * BLK
                for h in (0, 1):
                    ps1 = psum.tile([128, BLK], F32, tag="ps1")
                    nc.tensor.matmul(
                        out=ps1[:],
                        lhsT=w1b[64 * h:64 * h + 64, :],
                        rhs=s1b[64 * h:64 * h + 64, kb0:kb1],
                        start=True,
                        stop=True,
                    )
                    h1s = pool.tile([128, BLK], BF16, tag="h1s")
                    nc.scalar.activation(
                        h1s[:],
                        ps1[:],
                        mybir.ActivationFunctionType.Relu,
                        bias=b1s[:],
                    )
                    ps2 = psum.tile([8, BLK], F32, tag="ps2")
                    nc.tensor.matmul(
                        out=ps2[:], lhsT=w2b[:], rhs=h1s[:], start=True, stop=True
                    )
                    nc.vector.tensor_tensor(
                        out=q2sb[:, h * K2 + kb0:h * K2 + kb1],
                        in0=ps2[:],
                        in1=disqs[:, h * K2 + kb0:h * K2 + kb1],
                        op=mybir.AluOpType.mult,
                    )
            nc.sync.dma_start(out=q2d[:], in_=q2sb[:])
    nc.compile()
    return nc


def build_p3(caps3):
    """out = dis*(sum_d mg2) + b2 per core (self-loop is slot 0)."""
    coloff = np.zeros(K3 + 1, np.int64)
    np.cumsum(caps3, out=coloff[1:])
    S3 = int(coloff[K3])

    nc = bacc.Bacc("TRN2", target_bir_lowering=False, debug=False)
    mg2 = nc.dram_tensor("mg2", [128, 2 * S3], BF16, kind="ExternalInput")
    disl3 = nc.dram_tensor("disl3", [128, K3], F32, kind="ExternalInput")
    b2r = nc.dram_tensor("b2r", [128, D2], F32, kind="ExternalInput")
    out3 = nc.dram_tensor("out3", [128, K3 * D2], F32, kind="ExternalOutput")

    with tile.TileContext(nc) as tc:
        with tc.tile_pool(name="pool", bufs=2) as pool, tc.tile_pool(
            name="cpool", bufs=1
        ) as cpool:
            disls = cpool.tile([128, K3], F32)
            nc.sync.dma_start(out=disls[:], in_=disl3[:])
            b2s = cpool.tile([128, D2], F32)
            nc.sync.dma_start(out=b2s[:], in_=b2r[:])
            NCH = 2
            KC = K3 // NCH
            for u in range(NCH):
                ku0, ku1 = u * KC, (u + 1) * KC
                c0, c1 = 2 * int(coloff[ku0]), 2 * int(coloff[ku1])
                mgt = pool.tile([128, c1 - c0], BF16, tag="mgt")
                nc.sync.dma_start(out=mgt[:], in_=mg2[:, c0:c1])
                agg = pool.tile([128, KC * D2], F32, tag="agg")
                for k0, k1, cap in _runs(caps3[ku0:ku1]):
                    o0 = 2 * int(coloff[ku0 + k0]) - c0
                    o1 = 2 * int(coloff[ku0 + k1]) - c0
                    nc.vector.tensor_reduce(
                        out=agg[:, k0 * D2:k1 * D2],
                        in_=mgt[:, o0:o1].rearrange(
                            "p (k f d) -> p k f d", f=D2, d=cap
                        ),
                        axis=mybir.AxisListType.X,
                        op=mybir.AluOpType.add,
                    )
                nc.vector.tensor_tensor(
                    out=agg[:].rearrange("p (k f) -> p k f", f=D2),
                    in0=agg[:].rearrange("p (k f) -> p k f", f=D2),
                    in1=disls[:, ku0:ku1]
                    .rearrange("p (k o) -> p k o", o=1)
                    .to_broadcast([128, KC, D2]),
                    op=mybir.AluOpType.mult,
                )
                nc.vector.tensor_tensor(
                    out=agg[:].rearrange("p (k f) -> p k f", f=D2),
                    in0=agg[:].rearrange("p (k f) -> p k f", f=D2),
                    in1=b2s[:]
                    .rearrange("p (o f) -> p o f", o=1)
                    .to_broadcast([128, KC, D2]),
                    op=mybir.AluOpType.add,
                )
                nc.sync.dma_start(
                    out=out3[:, ku0 * D2:ku1 * D2], in_=agg[:]
                )
    nc.compile()
    return nc


def kernel(x, edge_index, W1, b1, W2, b2):
    LAST_EXEC_NS.clear()
    x = np.asarray(x, np.float32)
    W1 = np.asarray(W1, np.float32)
    b1 = np.asarray(b1, np.float32)
    W2 = np.asarray(W2, np.float32)
    b2 = np.asarray(b2, np.float32)
    src = np.asarray(edge_index[0], np.int64)
    dst = np.asarray(edge_index[1], np.int64)

    # ---- host index prep: dst-sorted slot table (self first) ----
    deg = np.bincount(dst, minlength=N).astype(np.int64)
    capmax = _ceil2(int(deg.max()) + 1)
    order_e = np.argsort(dst, kind="stable")
    s_src = src[order_e]
    s_dst = dst[order_e]
    starts = np.zeros(N + 1, np.int64)
    np.cumsum(deg, out=starts[1:])
    slot = np.full((N + 1, capmax), N, np.int64)  # row N = sentinel
    slot[:N, 0] = np.arange(N)                   # self-loop slot
    pos = np.arange(len(s_src)) - starts[s_dst]
    slot[s_dst, pos + 1] = s_src

    onode = np.argsort(-deg, kind="stable")
    order_ext = np.concatenate([onode, np.full(N2PAD - N, N, np.int64)])
    deg_ext = np.concatenate([deg[onode] + 1, np.ones(N2PAD - N, np.int64)])
    caps2 = np.maximum(_ceil2(deg_ext[::64]), 2).astype(np.int64)    # [K2]
    caps3 = np.maximum(_ceil2(deg_ext[::1024]), 2).astype(np.int64)  # [K3]
    nodes2 = order_ext.reshape(K2, 8, NCORES)    # [k, g, c]
    nodes3 = order_ext.reshape(K3, NCORES, 128)  # [k, c, p]
    runs2 = _merged_runs(caps2)
    runs3 = _runs(caps3)
    coloff3 = np.zeros(K3 + 1, np.int64)
    np.cumsum(caps3, out=coloff3[1:])

    # ---- P1: dis + q0(bf16) on 8 cores ----
    p1 = build_p1()
    in1 = []
    for c in range(NCORES):
        xp = np.zeros((P1PAD, D0), np.float32)
        xp[:NPC1] = x[c * NPC1:(c + 1) * NPC1]
        dg = np.zeros(P1PAD, np.float32)
        dg[:NPC1] = deg[c * NPC1:(c + 1) * NPC1]
        in1.append(
            {
                "xc": np.ascontiguousarray(xp.reshape(128, P1K * D0)),
                "degc": np.ascontiguousarray(dg.reshape(128, P1K)),
            }
        )
    r1 = _run(p1, in1, core_ids=CORE_IDS).results
    q0ext = np.zeros((N + 1, D0), NPBF)
    dis = np.zeros(N + 1, np.float32)
    disb = np.zeros(N + 1, NPBF)
    for c in range(NCORES):
        sl = slice(c * NPC1, (c + 1) * NPC1)
        q0ext[sl] = np.asarray(r1[c]["q0o"]).reshape(P1PAD, D0)[:NPC1]
        dis[sl] = np.asarray(r1[c]["diso"]).reshape(P1PAD)[:NPC1]
        disb[sl] = np.asarray(r1[c]["disbo"]).reshape(P1PAD)[:NPC1]

    # ---- host join 1: d-major bucketed mg1 slots ----
    p2 = build_p2(runs2)
    smg = sum((k1 - k0) * v for k0, k1, v in runs2)
    w1blk = np.zeros((128, 128), np.float32)
    w2blk = np.zeros((128, 8), np.float32)
    for g in range(4):
        w1blk[16 * g:16 * g + 16, 32 * g:32 * g + 32] = W1
        w1blk[64 + 16 * g:64 + 16 * g + 16, 32 * g:32 * g + 32] = W1
        w2blk[32 * g:32 * g + 32, 2 * g:2 * g + 2] = W2
    b1blk = np.ascontiguousarray(np.tile(b1, 4).reshape(128, 1))
    in2 = []
    for c in range(NCORES):
        grid = nodes2[:, :, c].T  # [8, K2]
        mg1 = np.empty((128, smg), NPBF)
        off = 0
        for k0, k1, v in runs2:
            L = k1 - k0
            idx = slot[grid[:, k0:k1], :v]          # [8, L, v]
            g = q0ext[idx.transpose(0, 2, 1)]       # [8, v, L, 16]
            mg1[:, off:off + L * v] = g.transpose(0, 3, 1, 2).reshape(128, -1)
            off += L * v
        disg = disb[grid]  # [8, K2] bf16
        disfc = np.ascontiguousarray(
            np.repeat(disg[:, None, :], 16, axis=1)
        ).reshape(128, K2)
        # disq rows r = 2*g' + j, cols h*K2 + k hold dis(node(4h+g', k))
        disqc = np.ascontiguousarray(
            np.repeat(
                dis[grid].reshape(2, 4, 1, K2).transpose(1, 2, 0, 3), 2, axis=1
            ).reshape(8, 2 * K2)
        )
        in2.append(
            {
                "mg1": mg1,
                "disf": disfc,
                "disq": disqc,
                "w1d": w1blk,
                "w2d": w2blk,
                "b1d": b1blk,
            }
        )
    r2 = _run(p2, in2, core_ids=CORE_IDS).results
    q2ext = np.zeros((N + 1, D2), NPBF)
    for c in range(NCORES):
        vals = (
            np.asarray(r2[c]["q2d"])
            .reshape(4, 2, 2, K2)       # [g', j, h, k]
            .transpose(2, 0, 3, 1)      # [h, g', k, j]
            .reshape(8, K2, 2)
        )
        q2ext[nodes2[:, :, c].T] = vals  # [8, K2, 2]
    q2ext[N] = 0

    # ---- host join 2: mg2 slots ----
    p3 = build_p3(caps3)
    S3 = int(coloff3[K3])
    b2r = np.ascontiguousarray(np.tile(b2.reshape(1, D2), (128, 1)))
    in3 = []
    for c in range(NCORES):
        grid = nodes3[:, c, :].T  # [128, K3]
        mg2 = np.empty((128, 2 * S3), NPBF)
        for k0, k1, cap in runs3:
            g = q2ext[slot[grid[:, k0:k1], :cap]]  # [128, L, cap, 2]
            mg2[:, 2 * coloff3[k0]:2 * coloff3[k1]] = g.transpose(
                0, 1, 3, 2
            ).reshape(128, -1)
        dislc = np.ascontiguousarray(dis[grid])
        in3.append({"mg2": mg2, "disl3": dislc, "b2r": b2r})
    r3 = _run(p3, in3, core_ids=CORE_IDS).results
    outfull = np.zeros((N + 1, D2), np.float32)
    for c in range(NCORES):
        outfull[nodes3[:, c, :].T] = np.asarray(r3[c]["out3"]).reshape(
            128, K3, D2
        )
    return np.ascontiguousarray(outfull[:N])
